# revision 4
# baseline (speedup 1.0000x reference)
"""Multi-head attention (B=4, L=2048, D=1024, H=16) on 8 TRN2 NeuronCores.

Sharding: 8 cores = 4 batches x 2 query-halves. Each core computes the
complete output rows for its (batch, q-half): Q projection for its rows,
full K/V projections for its batch (duplicated across the core pair --
cheaper than any collective), all 16 heads of attention for its 1024
query rows, and the out projection. Output rows are disjoint; the host
concatenates. No collectives.

v2 changes vs v1:
  - x^T produced by cast-to-bf16 (DVE) + 2-byte transpose-DMA
    (SBUF->SBUF) instead of PE transpose + PSUM copy: PE and PSUM freed.
  - all projections in bf16 (W cast fp32->bf16 on the otherwise-idle
    ScalarE), full-rate matmuls, half the SBUF.
  - mask prep (int32 -> bf16 -> transposed) emitted early, overlapping
    the projection phase instead of serializing after it.
  - one shared PSUM "wide" pool (2x [128,1024]) spans projections,
    scores, and out-projection; 4 ctx banks; no phase-boundary bubbles.
  - out projection as K=128 accumulation chains (heads pair-interleaved
    in partitions) -- one PSUM chain, single bias add.

Per-core attention pipeline (unchanged from v1):
  scores transposed ST[kp,q] = K^T.T @ Q^T per head-pair (K=64 row
  packing via base partition 0/64), exp on ScalarE from PSUM (bf16 out),
  mask as multiply on DVE (exp*0), ctx^T accumulated with an extra
  ones-column FIRST in V_aug so PSUM partition 0 is the softmax
  denominator; normalize via reciprocal + gpsimd partition_broadcast.
"""
import sys
import numpy as np

sys.path.insert(0, '/opt/trn_rl_repo')

import concourse.bass as bass
import concourse.mybir as mybir
from concourse import bacc
from concourse.tile import TileContext

F32 = mybir.dt.float32
BF16 = mybir.dt.bfloat16
I32 = mybir.dt.int32

B, L, D, H = 4, 2048, 1024, 16
HD = D // H            # 64
QL = L // 2            # 1024 q rows per core
KC = D // 128          # 8 contraction chunks of the model dim
KPC = L // 128         # 16 key-position chunks
NPAIR = H // 2         # 8 head pairs
SCALE = 1.0 / float(np.sqrt(HD))


def build_nc(debug_stage=None):
    nc = bacc.Bacc(None, target_bir_lowering=False)

    xq = nc.declare_dram_parameter("xq", [QL, D], F32, isOutput=False)
    xk = nc.declare_dram_parameter("xk", [L, D], F32, isOutput=False)
    xv = nc.declare_dram_parameter("xv", [L, D], F32, isOutput=False)
    maskq = nc.declare_dram_parameter("maskq", [QL, L], I32, isOutput=False)
    Wd, bd = {}, {}
    for nm in ("WQ", "WK", "WV", "WO"):
        Wd[nm] = nc.declare_dram_parameter(nm, [D, D], F32, isOutput=False)
    for nm in ("bQ", "bK", "bV", "bO"):
        bd[nm] = nc.declare_dram_parameter(nm, [D], F32, isOutput=False)
    out = nc.declare_dram_parameter("out", [QL, D], F32, isOutput=True)

    with TileContext(nc, pool_alloc_mode="queue") as tc:
        with tc.tile_pool(name="big", bufs=1) as big, \
             tc.tile_pool(name="const", bufs=1) as constp, \
             tc.tile_pool(name="wide", bufs=2, space="PSUM") as wide:
            bQ_sb = constp.tile([128, KC], F32)
            bK_sb = constp.tile([128, KC], F32)
            nc.sync.dma_start(bQ_sb, bd["bQ"].rearrange("(c p) -> p c", p=128))
            nc.sync.dma_start(bK_sb, bd["bK"].rearrange("(c p) -> p c", p=128))
            bV_bc = constp.tile([128, D], F32)
            nc.sync.dma_start(
                bV_bc,
                bd["bV"].rearrange("(o d) -> o d", o=1).partition_broadcast(128)[:, 0])

            # resident activation state
            QT = big.tile([128, KC, QL], BF16)     # [do%128, do//128, q]
            KT = big.tile([128, KC, L], BF16)      # [do%128, do//128, kp]
            Vaug = big.tile([128, KPC, H * (HD + 1)], BF16)
            Vaug_r = Vaug.rearrange("p k (h c) -> p k h c", c=HD + 1)
            mT = big.tile([128, KPC, QL], BF16)    # transposed 0/1 mask
            ctxP = big.tile([128, NPAIR, QL], BF16)  # pair-stacked ctx^T

            # ---- projections (bf16) + mask prep, one overlapped phase ----
            with tc.tile_pool(name="wbf", bufs=1) as wbfp, \
                 tc.tile_pool(name="wst", bufs=2) as wstage, \
                 tc.tile_pool(name="xt", bufs=2) as xtp, \
                 tc.tile_pool(name="xst", bufs=2) as xstage, \
                 tc.tile_pool(name="xbs", bufs=2) as xbstage, \
                 tc.tile_pool(name="mk", bufs=2) as mkp:

                def load_w_bf16(w_dram):
                    """DMA W fp32 chunkwise, cast to bf16 on ScalarE."""
                    w = wbfp.tile([128, KC, D], BF16, tag="w")
                    wr = w_dram.rearrange("(c p) m -> p c m", p=128)
                    for c in range(KC):
                        wf = wstage.tile([128, D], F32, tag="wf")
                        nc.sync.dma_start(wf, wr[:, c])
                        nc.scalar.copy(w[:, c], wf)
                    return w

                def transpose_slab_dma(x_slab):
                    """x_slab [1024, D] fp32 DRAM -> x^T [128, KC, 1024] bf16
                    via DVE cast + 2-byte transpose-DMA (no PE, no PSUM)."""
                    xT = xtp.tile([128, KC, 1024], BF16, tag="xT")
                    for rc in range(8):
                        xin = xstage.tile([128, D], F32, tag="xin")
                        nc.sync.dma_start(xin, x_slab[rc * 128:(rc + 1) * 128, :])
                        xb = xbstage.tile([128, D], BF16, tag="xb")
                        nc.vector.tensor_copy(xb, xin)
                        nc.sync.dma_start_transpose(
                            xT[:, :, rc * 128:(rc + 1) * 128], xb)
                    return xT

                # Q projection: QT[do, q]
                wq = load_w_bf16(Wd["WQ"])
                xqT = transpose_slab_dma(xq)
                for m in range(KC):
                    ps = wide.tile([128, 1024], F32, tag="ps")
                    for k in range(KC):
                        for n2 in range(2):
                            nc.tensor.matmul(
                                ps[:, n2 * 512:(n2 + 1) * 512],
                                wq[:, k, m * 128:(m + 1) * 128],
                                xqT[:, k, n2 * 512:(n2 + 1) * 512],
                                start=(k == 0), stop=(k == KC - 1))
                    nc.vector.tensor_scalar_add(
                        QT[:, m, :], ps, bQ_sb[:, m:m + 1])

                # V projection (natural layout) into V_aug; ones-column
                # FIRST so the ctx matmul row-sum lands at PSUM partition 0
                nc.vector.memset(Vaug_r[:, :, :, 0], 1.0)
                wv = load_w_bf16(Wd["WV"])
                for sl in range(2):
                    xvT = transpose_slab_dma(xv[sl * 1024:(sl + 1) * 1024, :])
                    for m in range(KC):
                        kpc = sl * 8 + m
                        ps = wide.tile([128, 1024], F32, tag="ps")
                        for k in range(KC):
                            for n2 in range(2):
                                nc.tensor.matmul(
                                    ps[:, n2 * 512:(n2 + 1) * 512],
                                    xvT[:, k, m * 128:(m + 1) * 128],
                                    wv[:, k, n2 * 512:(n2 + 1) * 512],
                                    start=(k == 0), stop=(k == KC - 1))
                        for n2 in range(2):
                            nc.vector.tensor_add(
                                Vaug_r[:, kpc, n2 * 8:(n2 + 1) * 8, 1:HD + 1],
                                ps[:, n2 * 512:(n2 + 1) * 512]
                                .rearrange("p (h d) -> p h d", d=HD),
                                bV_bc[:, n2 * 512:(n2 + 1) * 512]
                                .rearrange("p (h d) -> p h d", d=HD))

                # mask prep: int32 [q, kp] -> bf16 0/1 transposed to [kp, q];
                # emitted here so its DMA+DVE work overlaps the K projection
                mq = maskq.rearrange("(c p) l -> p c l", p=128)
                for c in range(KC):
                    for h2 in range(4):
                        sl_l = slice(h2 * 512, (h2 + 1) * 512)
                        mi = mkp.tile([128, 512], I32, tag="mi")
                        nc.sync.dma_start(mi, mq[:, c, sl_l])
                        mb = mkp.tile([128, 512], BF16, tag="mb")
                        nc.vector.tensor_copy(mb, mi)
                        nc.sync.dma_start_transpose(
                            mT[:, h2 * 4:(h2 + 1) * 4,
                               c * 128:(c + 1) * 128], mb)

                # K projection: KT[do, kp]
                wk = load_w_bf16(Wd["WK"])
                for sl in range(2):
                    xkT = transpose_slab_dma(xk[sl * 1024:(sl + 1) * 1024, :])
                    for m in range(KC):
                        ps = wide.tile([128, 1024], F32, tag="ps")
                        for k in range(KC):
                            for n2 in range(2):
                                nc.tensor.matmul(
                                    ps[:, n2 * 512:(n2 + 1) * 512],
                                    wk[:, k, m * 128:(m + 1) * 128],
                                    xkT[:, k, n2 * 512:(n2 + 1) * 512],
                                    start=(k == 0), stop=(k == KC - 1))
                        nc.vector.tensor_scalar_add(
                            KT[:, m, sl * 1024:(sl + 1) * 1024],
                            ps, bK_sb[:, m:m + 1])

            if debug_stage == "proj":
                with tc.tile_pool(name="dbg", bufs=1) as dbgp:
                    dbg = dbgp.tile([128, D], F32)
                    nc.vector.tensor_copy(dbg, KT[:, 0, 0:D])
                    nc.sync.dma_start(out[0:128, :], dbg)
                    dbg2 = dbgp.tile([128, 8, 128], F32)
                    nc.vector.tensor_copy(dbg2, Vaug[:, 0:8, 0:128])
                    nc.sync.dma_start(
                        out[128:256, :],
                        dbg2.rearrange("p a b -> p (a b)"))

            # ---- attention + out projection ----
            if debug_stage not in ("proj", "mask"):
              with tc.tile_pool(name="cx", bufs=1, space="PSUM") as psum_cx, \
                   tc.tile_pool(name="pb", bufs=4) as pbp, \
                   tc.tile_pool(name="nr", bufs=2) as nrp, \
                   tc.tile_pool(name="ow", bufs=1) as owp, \
                   tc.tile_pool(name="os", bufs=2) as osp:
                  # out-proj weights: WO rows j*128..(j+1)*128 are exactly
                  # heads 2j,2j+1 -> matches ctxP partition interleave
                  bO_bc = owp.tile([128, D], F32)
                  nc.sync.dma_start(
                      bO_bc,
                      bd["bO"].rearrange("(o d) -> o d", o=1).partition_broadcast(128)[:, 0])
                  wo = owp.tile([128, NPAIR, D], BF16)
                  for j in range(NPAIR):
                      wf = osp.tile([128, D], F32, tag="wf")
                      nc.sync.dma_start(
                          wf, Wd["WO"][j * 128:(j + 1) * 128, :])
                      nc.vector.tensor_copy(wo[:, j], wf)

                  for p in range(NPAIR):
                      cps = [psum_cx.tile([HD + 1, 512], F32, tag=f"cps{i}",
                                          name=f"cps{i}")
                             for i in range(4)]
                      for kpc in range(KPC):
                          scs, pms = [], []
                          for hl in range(2):
                              lo = hl * 64
                              sc = wide.tile([128, 1024], F32, tag="ps",
                                             name="sc")
                              scs.append(sc)
                              lhsT = KT[lo:lo + 64, p, kpc * 128:(kpc + 1) * 128]
                              for qh in range(2):
                                  nc.tensor.matmul(
                                      sc[:, qh * 512:(qh + 1) * 512], lhsT,
                                      QT[lo:lo + 64, p, qh * 512:(qh + 1) * 512],
                                      start=True, stop=True)
                          for hl in range(2):
                              pm = pbp.tile([128, 1024], BF16, tag="pm",
                                            name="pm")
                              pms.append(pm)
                              nc.scalar.activation(
                                  pm, scs[hl],
                                  mybir.ActivationFunctionType.Exp, scale=SCALE)
                          for hl in range(2):
                              nc.vector.tensor_mul(pms[hl], pms[hl],
                                                   mT[:, kpc, :])
                          for hl in range(2):
                              h = 2 * p + hl
                              for qh in range(2):
                                  nc.tensor.matmul(
                                      cps[hl * 2 + qh],
                                      Vaug[:, kpc, h * 65:(h + 1) * 65],
                                      pms[hl][:, qh * 512:(qh + 1) * 512],
                                      start=(kpc == 0), stop=(kpc == KPC - 1))
                      for hl in range(2):
                          ctmp = nrp.tile([65, QL], BF16, tag="ctmp")
                          for qh in range(2):
                              ps = cps[hl * 2 + qh]
                              srec = nrp.tile([128, 512], F32, tag="srec")
                              rep = nrp.tile([65, 512], F32, tag="rep")
                              nc.vector.reciprocal_approx_fast(
                                  srec[0:1, :], ps[0:1, :])
                              nc.gpsimd.partition_broadcast(
                                  rep, srec[0:1, :], channels=65)
                              nc.vector.tensor_mul(
                                  ctmp[:, qh * 512:(qh + 1) * 512],
                                  ps, rep)
                          nc.sync.dma_start(
                              ctxP[hl * 64:hl * 64 + 64, p, :],
                              ctmp[1:65, :])

                  if debug_stage == "attn":
                      with tc.tile_pool(name="dbg", bufs=1) as dbgp:
                          for j in range(H):
                              dbg = dbgp.tile([64, QL], F32, tag="dbg")
                              nc.vector.tensor_copy(dbg, ctxP[:, j, :])
                              nc.sync.dma_start(
                                  out[j * 64:(j + 1) * 64, :], dbg)

                  # out projection: K=128 chains (pair-interleaved heads)
                  if debug_stage != "attn":
                      for m in range(KC):          # q chunks
                          ps = wide.tile([128, 1024], F32, tag="ps",
                                         name="ops")
                          for j in range(NPAIR):
                              for n2 in range(2):
                                  nc.tensor.matmul(
                                      ps[:, n2 * 512:(n2 + 1) * 512],
                                      ctxP[:, j, m * 128:(m + 1) * 128],
                                      wo[:, j, n2 * 512:(n2 + 1) * 512],
                                      start=(j == 0), stop=(j == NPAIR - 1))
                          ot = osp.tile([128, 1024], F32, tag="ot")
                          nc.vector.tensor_add(ot, ps, bO_bc)
                          nc.sync.dma_start(out[m * 128:(m + 1) * 128, :], ot)

    nc.compile()
    return nc


_NC = None


def _get_nc():
    global _NC
    if _NC is None:
        _NC = build_nc()
    return _NC


def make_in_maps(q, k, v, mask, WQ, bQ, WK, bK, WV, bV, WO, bO):
    in_maps = []
    for c in range(8):
        b, qh = c // 2, c % 2
        sl = slice(qh * QL, (qh + 1) * QL)
        in_maps.append({
            "xq": np.ascontiguousarray(q[b, sl]),
            "xk": np.ascontiguousarray(k[b]),
            "xv": np.ascontiguousarray(v[b]),
            "maskq": np.ascontiguousarray(mask[b, 0, sl]),
            "WQ": WQ, "WK": WK, "WV": WV, "WO": WO,
            "bQ": bQ, "bK": bK, "bV": bV, "bO": bO,
        })
    return in_maps


def assemble_output(res):
    outp = np.empty((B, L, D), np.float32)
    for c in range(8):
        b, qh = c // 2, c % 2
        outp[b, qh * QL:(qh + 1) * QL] = res.results[c]["out"]
    return outp


def kernel(q, k, v, mask, WQ, bQ, WK, bK, WV, bV, WO, bO):
    from concourse.bass_utils import run_bass_kernel_spmd
    q = np.asarray(q, np.float32)
    k = np.asarray(k, np.float32)
    v = np.asarray(v, np.float32)
    mask = np.asarray(mask, np.int32)
    args = [np.asarray(a, np.float32) for a in (WQ, bQ, WK, bK, WV, bV, WO, bO)]
    nc = _get_nc()
    in_maps = make_in_maps(q, k, v, mask, *args)
    res = run_bass_kernel_spmd(nc, in_maps, list(range(8)))
    return assemble_output(res)


# revision 9
# speedup vs baseline: 1.0300x; 1.0300x over previous
"""Multi-head attention (B=4, L=2048, D=1024, H=16) on 8 TRN2 NeuronCores.

Sharding: 8 cores = 4 batches x 2 query-halves. Each core computes the
complete output rows for its (batch, q-half). K/V projections are
duplicated across the core pair (cheaper than any collective); output
rows are disjoint so the host just concatenates.

v3 structure (vs v1 baseline):
  - bf16 projections: W cast fp32->bf16 on the otherwise-idle ScalarE,
    x^T cast to bf16 in the PSUM->SBUF copy after the PE transpose.
  - phase order Q-proj, V-proj (+mask prep overlapped), then K-proj
    emitted m-chunk-by-m-chunk INTERLEAVED with attention pairs: pair p
    only needs K^T chunk m=p, so exp starts ~150us earlier and the K
    matmuls fill PE slack under the ACT-bound attention phase.
  - one shared PSUM pool (2x [128,1024]) carries proj chains, scores,
    and out-proj; transpose PSUM (4 banks) closes before ctx PSUM opens.
  - ctx PSUM banks released early: unnormalized ctx^T + denominator row
    are DMA'd PSUM->SBUF, and reciprocal/broadcast/multiply run off the
    critical path; the next pair's ctx chains start ~6us sooner.
  - out projection as K=128 accumulation chains (heads pair-interleaved
    in partitions), single bias add.

Attention math per pair (unchanged): transposed scores ST[kp,q] =
K^T.T @ Q^T (K=64 row packing, base partition 0/64), exp on ScalarE
(scale=1/sqrt(64)), mask as bf16 multiply on DVE (exp * 0), ctx^T
accumulated with a ones-column FIRST in V_aug so PSUM partition 0 is
the softmax denominator.
"""
import sys
import numpy as np

sys.path.insert(0, '/opt/trn_rl_repo')

import concourse.bass as bass
import concourse.mybir as mybir
from concourse import bacc
from concourse.tile import TileContext
from concourse.masks import make_identity

F32 = mybir.dt.float32
BF16 = mybir.dt.bfloat16
I32 = mybir.dt.int32

B, L, D, H = 4, 2048, 1024, 16
HD = D // H            # 64
QL = L // 2            # 1024 q rows per core
KC = D // 128          # 8 contraction chunks of the model dim
KPC = L // 128         # 16 key-position chunks
NPAIR = H // 2         # 8 head pairs
SCALE = 1.0 / float(np.sqrt(HD))


def build_nc(debug_stage=None):
    nc = bacc.Bacc(None, target_bir_lowering=False)

    xq = nc.declare_dram_parameter("xq", [QL, D], F32, isOutput=False)
    xk = nc.declare_dram_parameter("xk", [L, D], F32, isOutput=False)
    xv = nc.declare_dram_parameter("xv", [L, D], F32, isOutput=False)
    maskq = nc.declare_dram_parameter("maskq", [QL, L], I32, isOutput=False)
    Wd, bd = {}, {}
    for nm in ("WQ", "WK", "WV", "WO"):
        Wd[nm] = nc.declare_dram_parameter(nm, [D, D], F32, isOutput=False)
    for nm in ("bQ", "bK", "bV", "bO"):
        bd[nm] = nc.declare_dram_parameter(nm, [D], F32, isOutput=False)
    out = nc.declare_dram_parameter("out", [QL, D], F32, isOutput=True)

    with TileContext(nc, pool_alloc_mode="queue") as tc:
        with tc.tile_pool(name="big", bufs=1) as big, \
             tc.tile_pool(name="const", bufs=1) as constp, \
             tc.tile_pool(name="wide", bufs=2, space="PSUM") as wide:
            ident = constp.tile([128, 128], F32)
            make_identity(nc, ident)
            bQ_sb = constp.tile([128, KC], F32)
            bK_sb = constp.tile([128, KC], F32)
            nc.sync.dma_start(bQ_sb, bd["bQ"].rearrange("(c p) -> p c", p=128))
            nc.sync.dma_start(bK_sb, bd["bK"].rearrange("(c p) -> p c", p=128))
            bV_bc = constp.tile([128, D], F32)
            nc.sync.dma_start(
                bV_bc,
                bd["bV"].rearrange("(o d) -> o d", o=1).partition_broadcast(128)[:, 0])

            # resident activation state
            QT = big.tile([128, KC, QL], BF16)     # [do%128, do//128, q]
            KT = big.tile([128, KC, L], BF16)      # [do%128, do//128, kp]
            Vaug = big.tile([128, KPC, H * (HD + 1)], BF16)
            Vaug_r = Vaug.rearrange("p k (h c) -> p k h c", c=HD + 1)
            mT = big.tile([128, KPC, QL], BF16)    # transposed 0/1 mask
            ctxP = big.tile([128, NPAIR, QL], BF16)  # pair-stacked ctx^T

            def load_w_bf16(pool, w_dram, stage_pool):
                """DMA W fp32 chunkwise, cast to bf16 on ScalarE."""
                w = pool.tile([128, KC, D], BF16, tag="w")
                wr = w_dram.rearrange("(c p) m -> p c m", p=128)
                for c in range(KC):
                    wf = stage_pool.tile([128, D], F32, tag="wf")
                    nc.sync.dma_start(wf, wr[:, c])
                    nc.scalar.copy(w[:, c], wf)
                return w

            # ---- phase 1: Q proj, V proj, mask prep ----
            with tc.tile_pool(name="wbf1", bufs=1) as wbf1, \
                 tc.tile_pool(name="wst1", bufs=2) as wst1, \
                 tc.tile_pool(name="xt1", bufs=2) as xt1, \
                 tc.tile_pool(name="xst1", bufs=2) as xst1, \
                 tc.tile_pool(name="mk", bufs=2) as mkp, \
                 tc.tile_pool(name="pt1", bufs=2, space="PSUM") as pt1:

                def transpose_slab(x_slab, xtp, xstage, psum_t):
                    """x [1024, D] fp32 DRAM -> x^T [128, KC, 1024] bf16
                    via PE transpose + DVE cast-copy."""
                    xT = xtp.tile([128, KC, 1024], BF16, tag="xT")
                    for rc in range(8):
                        xin = xstage.tile([128, D], F32, tag="xin")
                        nc.sync.dma_start(xin, x_slab[rc * 128:(rc + 1) * 128, :])
                        ps = psum_t.tile([128, 1024], F32, tag="pst")
                        for dc in range(KC):
                            nc.tensor.transpose(
                                ps[:, dc * 128:(dc + 1) * 128],
                                xin[:, dc * 128:(dc + 1) * 128], ident)
                        nc.vector.tensor_copy(
                            xT[:, :, rc * 128:(rc + 1) * 128],
                            ps.rearrange("p (c j) -> p c j", j=128))
                    return xT

                # Q projection: QT[do, q]
                wq = load_w_bf16(wbf1, Wd["WQ"], wst1)
                xqT = transpose_slab(xq, xt1, xst1, pt1)
                for m in range(KC):
                    ps = wide.tile([128, 1024], F32, tag="ps")
                    for k in range(KC):
                        for n2 in range(2):
                            nc.tensor.matmul(
                                ps[:, n2 * 512:(n2 + 1) * 512],
                                wq[:, k, m * 128:(m + 1) * 128],
                                xqT[:, k, n2 * 512:(n2 + 1) * 512],
                                start=(k == 0), stop=(k == KC - 1))
                    nc.vector.tensor_scalar_add(
                        QT[:, m, :], ps, bQ_sb[:, m:m + 1])

                # V projection (natural layout) into V_aug
                nc.vector.memset(Vaug_r[:, :, :, 0], 1.0)
                wv = load_w_bf16(wbf1, Wd["WV"], wst1)
                for sl in range(2):
                    xvT = transpose_slab(
                        xv[sl * 1024:(sl + 1) * 1024, :], xt1, xst1, pt1)
                    for m in range(KC):
                        kpc = sl * 8 + m
                        ps = wide.tile([128, 1024], F32, tag="ps")
                        for k in range(KC):
                            for n2 in range(2):
                                nc.tensor.matmul(
                                    ps[:, n2 * 512:(n2 + 1) * 512],
                                    xvT[:, k, m * 128:(m + 1) * 128],
                                    wv[:, k, n2 * 512:(n2 + 1) * 512],
                                    start=(k == 0), stop=(k == KC - 1))
                        for n2 in range(2):
                            nc.vector.tensor_add(
                                Vaug_r[:, kpc, n2 * 8:(n2 + 1) * 8, 1:HD + 1],
                                ps[:, n2 * 512:(n2 + 1) * 512]
                                .rearrange("p (h d) -> p h d", d=HD),
                                bV_bc[:, n2 * 512:(n2 + 1) * 512]
                                .rearrange("p (h d) -> p h d", d=HD))

                # mask prep: int32 [q, kp] -> bf16 0/1 transposed to [kp, q]
                mq = maskq.rearrange("(c p) l -> p c l", p=128)
                for c in range(KC):
                    for h2 in range(4):
                        sl_l = slice(h2 * 512, (h2 + 1) * 512)
                        mi = mkp.tile([128, 512], I32, tag="mi")
                        nc.sync.dma_start(mi, mq[:, c, sl_l])
                        mb = mkp.tile([128, 512], BF16, tag="mb")
                        nc.vector.tensor_copy(mb, mi)
                        nc.sync.dma_start_transpose(
                            mT[:, h2 * 4:(h2 + 1) * 4,
                               c * 128:(c + 1) * 128], mb)

            if debug_stage == "proj":
                with tc.tile_pool(name="dbg", bufs=1) as dbgp:
                    dbg = dbgp.tile([128, D], F32)
                    nc.vector.tensor_copy(dbg, QT[:, 0, 0:QL])
                    nc.sync.dma_start(out[0:128, :], dbg)
                    dbg2 = dbgp.tile([128, 8, 128], F32)
                    nc.vector.tensor_copy(dbg2, Vaug[:, 0:8, 0:128])
                    nc.sync.dma_start(
                        out[128:256, :],
                        dbg2.rearrange("p a b -> p (a b)"))

            # ---- phase 2: K proj interleaved with attention pairs ----
            if debug_stage not in ("proj", "mask"):
              with tc.tile_pool(name="wbf2", bufs=1) as wbf2, \
                   tc.tile_pool(name="wst2", bufs=1) as wst2, \
                   tc.tile_pool(name="xt2", bufs=2) as xt2, \
                   tc.tile_pool(name="xst2", bufs=2) as xst2, \
                   tc.tile_pool(name="pb", bufs=3) as pbp, \
                   tc.tile_pool(name="nr", bufs=1) as nrp:

                  wk = load_w_bf16(wbf2, Wd["WK"], wst2)
                  with tc.tile_pool(name="pt2", bufs=2, space="PSUM") as pt2:
                      xkT = [transpose_slab(
                                 xk[sl * 1024:(sl + 1) * 1024, :],
                                 xt2, xst2, pt2)
                             for sl in range(2)]

                  def k_proj_chunk(m):
                      for sl in range(2):
                          ps = wide.tile([128, 1024], F32, tag="ps")
                          for k in range(KC):
                              for n2 in range(2):
                                  nc.tensor.matmul(
                                      ps[:, n2 * 512:(n2 + 1) * 512],
                                      wk[:, k, m * 128:(m + 1) * 128],
                                      xkT[sl][:, k, n2 * 512:(n2 + 1) * 512],
                                      start=(k == 0), stop=(k == KC - 1))
                          nc.vector.tensor_scalar_add(
                              KT[:, m, sl * 1024:(sl + 1) * 1024],
                              ps, bK_sb[:, m:m + 1])

                  with tc.tile_pool(name="cx", bufs=1, space="PSUM") as psum_cx:
                      for p in range(NPAIR):
                          k_proj_chunk(p)
                          cps = [psum_cx.tile([HD + 1, 512], F32, tag=f"cps{i}",
                                              name=f"cps{i}")
                                 for i in range(4)]
                          for kpc in range(KPC):
                              scs, pms = [], []
                              for hl in range(2):
                                  lo = hl * 64
                                  sc = wide.tile([128, 1024], F32, tag="ps",
                                                 name="sc")
                                  scs.append(sc)
                                  lhsT = KT[lo:lo + 64, p, kpc * 128:(kpc + 1) * 128]
                                  for qh in range(2):
                                      nc.tensor.matmul(
                                          sc[:, qh * 512:(qh + 1) * 512], lhsT,
                                          QT[lo:lo + 64, p, qh * 512:(qh + 1) * 512],
                                          start=True, stop=True)
                              for hl in range(2):
                                  pm = pbp.tile([128, 1024], BF16, tag="pm",
                                                name="pm")
                                  pms.append(pm)
                                  nc.scalar.activation(
                                      pm, scs[hl],
                                      mybir.ActivationFunctionType.Exp, scale=SCALE)
                              for hl in range(2):
                                  nc.vector.tensor_mul(pms[hl], pms[hl],
                                                       mT[:, kpc, :])
                              for hl in range(2):
                                  h = 2 * p + hl
                                  for qh in range(2):
                                      nc.tensor.matmul(
                                          cps[hl * 2 + qh],
                                          Vaug[:, kpc, h * 65:(h + 1) * 65],
                                          pms[hl][:, qh * 512:(qh + 1) * 512],
                                          start=(kpc == 0), stop=(kpc == KPC - 1))
                          # tail: free the ctx banks ASAP via a DVE copy,
                          # normalize from SBUF off the critical path
                          for hl in range(2):
                              for qh in range(2):
                                  ctn = nrp.tile([65, 512], F32, tag="ctn")
                                  nc.vector.tensor_copy(ctn, cps[hl * 2 + qh])
                                  srec = nrp.tile([128, 512], F32, tag="srec")
                                  rep = nrp.tile([65, 512], F32, tag="rep")
                                  nc.vector.reciprocal_approx_fast(
                                      srec[0:1, :], ctn[0:1, :])
                                  nc.gpsimd.partition_broadcast(
                                      rep, srec[0:1, :], channels=65)
                                  ctmp = nrp.tile([65, 512], BF16, tag="ctmp")
                                  nc.vector.tensor_mul(ctmp, ctn, rep)
                                  nc.sync.dma_start(
                                      ctxP[hl * 64:hl * 64 + 64, p,
                                           qh * 512:(qh + 1) * 512],
                                      ctmp[1:65, :])

                  if debug_stage == "attn":
                      with tc.tile_pool(name="dbg", bufs=1) as dbgp:
                          for j in range(H):
                              dbg = dbgp.tile([64, QL], F32, tag="dbg")
                              nc.vector.tensor_copy(dbg, ctxP[:, j, :])
                              nc.sync.dma_start(
                                  out[j * 64:(j + 1) * 64, :], dbg)

            # ---- out projection: K=128 chains ----
            if debug_stage not in ("proj", "mask", "attn"):
              with tc.tile_pool(name="ow", bufs=1) as owp, \
                   tc.tile_pool(name="os", bufs=2) as osp:
                  bO_bc = owp.tile([128, D], F32)
                  nc.sync.dma_start(
                      bO_bc,
                      bd["bO"].rearrange("(o d) -> o d", o=1).partition_broadcast(128)[:, 0])
                  # WO rows j*128..(j+1)*128 are exactly heads 2j,2j+1 ->
                  # matches ctxP partition interleave
                  wo = owp.tile([128, NPAIR, D], BF16)
                  for j in range(NPAIR):
                      wf = osp.tile([128, D], F32, tag="wf")
                      nc.sync.dma_start(
                          wf, Wd["WO"][j * 128:(j + 1) * 128, :])
                      nc.vector.tensor_copy(wo[:, j], wf)
                  for m in range(KC):          # q chunks
                      ps = wide.tile([128, 1024], F32, tag="ps", name="ops")
                      for j in range(NPAIR):
                          for n2 in range(2):
                              nc.tensor.matmul(
                                  ps[:, n2 * 512:(n2 + 1) * 512],
                                  ctxP[:, j, m * 128:(m + 1) * 128],
                                  wo[:, j, n2 * 512:(n2 + 1) * 512],
                                  start=(j == 0), stop=(j == NPAIR - 1))
                      ot = osp.tile([128, 1024], F32, tag="ot")
                      nc.vector.tensor_add(ot, ps, bO_bc)
                      nc.sync.dma_start(out[m * 128:(m + 1) * 128, :], ot)

    nc.compile()
    return nc


_NC = None


def _get_nc():
    global _NC
    if _NC is None:
        _NC = build_nc()
    return _NC


def make_in_maps(q, k, v, mask, WQ, bQ, WK, bK, WV, bV, WO, bO):
    in_maps = []
    for c in range(8):
        b, qh = c // 2, c % 2
        sl = slice(qh * QL, (qh + 1) * QL)
        in_maps.append({
            "xq": np.ascontiguousarray(q[b, sl]),
            "xk": np.ascontiguousarray(k[b]),
            "xv": np.ascontiguousarray(v[b]),
            "maskq": np.ascontiguousarray(mask[b, 0, sl]),
            "WQ": WQ, "WK": WK, "WV": WV, "WO": WO,
            "bQ": bQ, "bK": bK, "bV": bV, "bO": bO,
        })
    return in_maps


def assemble_output(res):
    outp = np.empty((B, L, D), np.float32)
    for c in range(8):
        b, qh = c // 2, c % 2
        outp[b, qh * QL:(qh + 1) * QL] = res.results[c]["out"]
    return outp


def kernel(q, k, v, mask, WQ, bQ, WK, bK, WV, bV, WO, bO):
    from concourse.bass_utils import run_bass_kernel_spmd
    q = np.asarray(q, np.float32)
    k = np.asarray(k, np.float32)
    v = np.asarray(v, np.float32)
    mask = np.asarray(mask, np.int32)
    args = [np.asarray(a, np.float32) for a in (WQ, bQ, WK, bK, WV, bV, WO, bO)]
    nc = _get_nc()
    in_maps = make_in_maps(q, k, v, mask, *args)
    res = run_bass_kernel_spmd(nc, in_maps, list(range(8)))
    return assemble_output(res)
